# revision 16
# baseline (speedup 1.0000x reference)
"""LoRA layer (x @ W.T + (x@A)@B + bias) on 8 trn2 NeuronCores.

Data-parallel: core b computes batch b's (2048, 4096) output slice.
Per-core device work is a single fused matmul: the (4096-deep) base
projection accumulates 32 K=128 fp32r steps into PSUM, then one extra
K=17 step adds the low-rank correction and bias ((x@A | 1) @ (B ; bias)).
x@A (rank 16, 0.4% of FLOPs) and all transposes are host-side prep.
"""
import numpy as np

import concourse.mybir as mybir
import concourse.tile as tile
from concourse import bacc
from concourse.bass_utils import run_bass_kernel_spmd

BATCH, SEQ, DIN, DOUT, RANK = 8, 2048, 4096, 4096, 16
N_CORES = 8

KT = DIN // 128          # 32 contraction tiles
M_BLK = 1024             # tokens resident per block
N_MBLK = SEQ // M_BLK    # 2 blocks
MT_PER_BLK = M_BLK // 128  # 8 m-tiles -> 8 PSUM banks
OT = DOUT // 512         # 8 output-column tiles
F32R = mybir.dt.float32r
F32 = mybir.dt.float32

_nc_cache = []


def _build(no_x_dma=False, no_wt_dma=False, no_lora=False, no_out=False,
           wt_bufs=6, out_bufs=6, x_bufs=KT + 2, x_in_first_o=True, reps=1):
    nc = bacc.Bacc("TRN2", target_bir_lowering=False, debug=False)
    xT = nc.dram_tensor("xT", [DIN, SEQ], F32R, kind="ExternalInput")
    wT = nc.dram_tensor("wT", [DIN, DOUT], F32R, kind="ExternalInput")
    xaT = nc.dram_tensor("xaT", [RANK + 1, SEQ], F32R, kind="ExternalInput")
    bB = nc.dram_tensor("bB", [RANK + 1, DOUT], F32R, kind="ExternalInput")
    out = nc.dram_tensor("out", [SEQ, DOUT], F32, kind="ExternalOutput")

    with tile.TileContext(nc) as tc:
        with (
            tc.tile_pool(name="xblk", bufs=x_bufs) as xpool,
            tc.tile_pool(name="wt", bufs=wt_bufs) as wpool,
            tc.tile_pool(name="lora", bufs=1) as lpool,
            tc.tile_pool(name="outp", bufs=out_bufs) as opool,
            tc.tile_pool(name="psum", bufs=8, space="PSUM") as ppool,
        ):
            xa_sb = lpool.tile([RANK + 1, SEQ], F32R, tag="xa")
            nc.sync.dma_start(xa_sb[:], xaT[:, :])
            bB_sb = lpool.tile([RANK + 1, DOUT], F32R, tag="bB")
            nc.sync.dma_start(bB_sb[:], bB[:, :])

            def load_x(xpool, k, m0):
                xt = xpool.tile([128, M_BLK], F32R, name="x", tag="x")
                if no_x_dma:
                    nc.sync.dma_start(xt[:, :4], xT[k * 128:(k + 1) * 128, :4])
                else:
                    nc.sync.dma_start(
                        xt[:], xT[k * 128:(k + 1) * 128, m0:m0 + M_BLK])
                return xt

            import contextlib
            rep_ctx = tc.For_i(0, reps, 1) if reps > 1 else contextlib.nullcontext()
            with rep_ctx:
              for blk in range(N_MBLK):
                  m0 = blk * M_BLK
                  xtiles = [None] * KT
                  if not x_in_first_o:
                      for k in range(KT):
                          xtiles[k] = load_x(xpool, k, m0)
                  for o in range(OT):
                      o0 = o * 512
                      psums = [ppool.tile([128, 512], F32, name="ps", tag="ps")
                               for _ in range(MT_PER_BLK)]
                      for k in range(KT):
                          if xtiles[k] is None:
                              xtiles[k] = load_x(xpool, k, m0)
                          wt = wpool.tile([128, 512], F32R, name="w", tag="w")
                          if no_wt_dma:
                              nc.sync.dma_start(
                                  wt[:, :4], wT[k * 128:(k + 1) * 128, :4])
                          else:
                              nc.sync.dma_start(
                                  wt[:], wT[k * 128:(k + 1) * 128, o0:o0 + 512])
                          for mt in range(MT_PER_BLK):
                              nc.tensor.matmul(
                                  psums[mt][:],
                                  xtiles[k][:, mt * 128:(mt + 1) * 128],
                                  wt[:],
                                  start=(k == 0), stop=(no_lora and k == KT - 1))
                      for mt in range(MT_PER_BLK):
                          ms = m0 + mt * 128
                          if not no_lora:
                              nc.tensor.matmul(
                                  psums[mt][:],
                                  xa_sb[:, ms:ms + 128],
                                  bB_sb[:, o0:o0 + 512],
                                  start=False, stop=True)
                          ot = opool.tile([128, 512], F32, name="o", tag="o")
                          nc.vector.tensor_copy(ot[:], psums[mt][:])
                          if not no_out:
                              nc.sync.dma_start(
                                  out[ms:ms + 128, o0:o0 + 512], ot[:])
    nc.compile()
    return nc


def _build_b(no_x_dma=False, no_wt_dma=False, no_lora=False, no_out=False,
             wt_bufs=6, out_bufs=6, x_bufs=KT + 2, reps=1):
    """Orientation B: W slices stationary, resident x moving, out computed
    transposed ([DOUT, SEQ]) and transposed back on host."""
    nc = bacc.Bacc("TRN2", target_bir_lowering=False, debug=False)
    xT = nc.dram_tensor("xT", [DIN, SEQ], F32R, kind="ExternalInput")
    wT = nc.dram_tensor("wT", [DIN, DOUT], F32R, kind="ExternalInput")
    xaT = nc.dram_tensor("xaT", [RANK + 1, SEQ], F32R, kind="ExternalInput")
    bB = nc.dram_tensor("bB", [RANK + 1, DOUT], F32R, kind="ExternalInput")
    outT = nc.dram_tensor("outT", [DOUT, SEQ], F32, kind="ExternalOutput")

    with tile.TileContext(nc) as tc:
        with (
            tc.tile_pool(name="xblk", bufs=x_bufs) as xpool,
            tc.tile_pool(name="wt", bufs=wt_bufs) as wpool,
            tc.tile_pool(name="lora", bufs=1) as lpool,
            tc.tile_pool(name="outp", bufs=out_bufs) as opool,
            tc.tile_pool(name="psum", bufs=8, space="PSUM") as ppool,
        ):
            xa_sb = lpool.tile([RANK + 1, SEQ], F32R, tag="xa")
            nc.sync.dma_start(xa_sb[:], xaT[:, :])
            bB_sb = lpool.tile([RANK + 1, DOUT], F32R, tag="bB")
            nc.sync.dma_start(bB_sb[:], bB[:, :])

            def load_x(k, m0):
                xt = xpool.tile([128, M_BLK], F32R, name="x", tag="x")
                nc.sync.dma_start(
                    xt[:], xT[k * 128:(k + 1) * 128, m0:m0 + M_BLK])
                return xt

            import contextlib
            rep_ctx = tc.For_i(0, reps, 1) if reps > 1 else contextlib.nullcontext()
            with rep_ctx:
              for half in range(N_MBLK):
                m0 = half * M_BLK
                xtiles = [None] * KT
                for og in range(OT):
                    og0 = og * 512
                    psums = [ppool.tile([128, 512], F32, name="ps", tag="ps")
                             for _ in range(8)]
                    for k in range(KT):
                        if xtiles[k] is None:
                            xtiles[k] = load_x(k, m0)
                        wt = wpool.tile([128, 512], F32R, name="w", tag="w")
                        nc.sync.dma_start(
                            wt[:], wT[k * 128:(k + 1) * 128, og0:og0 + 512])
                        for oi in range(4):
                            for mc in range(2):
                                nc.tensor.matmul(
                                    psums[oi * 2 + mc][:],
                                    wt[:, oi * 128:(oi + 1) * 128],
                                    xtiles[k][:, mc * 512:(mc + 1) * 512],
                                    start=(k == 0), stop=False)
                    for oi in range(4):
                        for mc in range(2):
                            nc.tensor.matmul(
                                psums[oi * 2 + mc][:],
                                bB_sb[:, og0 + oi * 128:og0 + (oi + 1) * 128],
                                xa_sb[:, m0 + mc * 512:m0 + (mc + 1) * 512],
                                start=False, stop=True)
                            ot = opool.tile([128, 512], F32, name="o", tag="o")
                            nc.vector.tensor_copy(ot[:], psums[oi * 2 + mc][:])
                            nc.sync.dma_start(
                                outT[og0 + oi * 128:og0 + (oi + 1) * 128,
                                     m0 + mc * 512:m0 + (mc + 1) * 512],
                                ot[:])
    nc.compile()
    return nc


BF16 = mybir.dt.bfloat16


def _build_c(reps=1, w_bufs=64, o_bufs=8,
             no_w_dma=False, no_x_dma=False, no_out=False, kt=None):
    """Variant C: bf16, W' = (W.T + A@B) folded host-side, bias via scalar
    engine during PSUM eviction. x fully resident in SBUF (16 MiB bf16); W
    streamed exactly once (32 MiB bf16). Iterations use 4 PSUM banks
    (2 o-tiles x 2 token-chunks) so eviction of one iteration overlaps the
    next iteration's matmuls on the other 4 banks — no PE stalls."""
    nc = bacc.Bacc("TRN2", target_bir_lowering=False, debug=False)
    xT = nc.dram_tensor("xT", [DIN, SEQ], BF16, kind="ExternalInput")
    wT = nc.dram_tensor("wT", [DIN, DOUT], BF16, kind="ExternalInput")
    biasR = nc.dram_tensor("biasR", [128, DOUT // 128], F32, kind="ExternalInput")
    outT = nc.dram_tensor("outT", [DOUT, SEQ], F32, kind="ExternalOutput")

    KT_ = kt if kt is not None else DIN // 128   # contraction tiles (diag: fewer)
    OG = DOUT // 256           # 16 output groups of 256 channels
    copy_f = mybir.ActivationFunctionType.Identity

    with tile.TileContext(nc) as tc:
        with (
            tc.tile_pool(name="xr", bufs=2 * KT_) as xpool,
            tc.tile_pool(name="wt", bufs=w_bufs) as wpool,
            tc.tile_pool(name="bias", bufs=1) as bpool,
            tc.tile_pool(name="outp", bufs=o_bufs) as opool,
            tc.tile_pool(name="psum", bufs=8, space="PSUM") as ppool,
        ):
            bias_sb = bpool.tile([128, DOUT // 128], F32, tag="bias")
            nc.sync.dma_start(bias_sb[:], biasR[:, :])

            def load_w(og):
                og0 = og * 256
                lst = []
                for k in range(KT_):
                    w = wpool.tile([128, 256], BF16, name="w", tag="w")
                    if no_w_dma:
                        nc.sync.dma_start(w[:, :4], wT[k * 128:(k + 1) * 128, og0:og0 + 4])
                    else:
                        nc.sync.dma_start(
                            w[:], wT[k * 128:(k + 1) * 128, og0:og0 + 256])
                    lst.append(w)
                return lst

            def load_x(k, mh):
                t = xpool.tile([128, 1024], BF16, name="x", tag="x")
                if no_x_dma:
                    nc.sync.dma_start(t[:, :4], xT[k * 128:(k + 1) * 128, :4])
                else:
                    nc.sync.dma_start(
                        t[:], xT[k * 128:(k + 1) * 128,
                                 mh * 1024:(mh + 1) * 1024])
                return t

            import contextlib
            rep_ctx = tc.For_i(0, reps, 1) if reps > 1 else contextlib.nullcontext()
            with rep_ctx:
                # Issue order drives the DMA queue. Interleave og0's W tiles
                # with mh0's x tiles in consumption order (w[k]+x[k] DMA
                # ~0.9us/k matches the PE's 0.85us/k), so the PE starts ~1us
                # in and chases the DMA through og0/mh0 with no long stall.
                wt_first = []
                xt = [[None] * KT_, [None] * KT_]
                for k in range(KT_):
                    w = wpool.tile([128, 256], BF16, name="w", tag="w")
                    if no_w_dma:
                        nc.sync.dma_start(w[:, :4], wT[k * 128:(k + 1) * 128, :4])
                    else:
                        nc.sync.dma_start(w[:], wT[k * 128:(k + 1) * 128, 0:256])
                    wt_first.append(w)
                    xt[0][k] = load_x(k, 0)
                for k in range(KT_):
                    xt[1][k] = load_x(k, 1)
                for og in range(OG):
                    og0 = og * 256
                    wt_t = wt_first if og == 0 else load_w(og)
                    for mh in range(2):
                        last_iter = (og == OG - 1 and mh == 1)
                        ps = [ppool.tile([128, 512], F32, name="ps", tag="ps")
                              for _ in range(4)]
                        if last_iter:
                            # Per-bank k-runs so evictions pipeline into the
                            # tail instead of all starting after the last MM.
                            for oi in range(2):
                                for mc in range(2):
                                    for k in range(KT_):
                                        nc.tensor.matmul(
                                            ps[oi * 2 + mc][:],
                                            wt_t[k][:, oi * 128:(oi + 1) * 128],
                                            xt[mh][k][:, mc * 512:(mc + 1) * 512],
                                            start=(k == 0), stop=(k == KT_ - 1))
                        else:
                            for k in range(KT_):
                                for oi in range(2):
                                    for mc in range(2):
                                        nc.tensor.matmul(
                                            ps[oi * 2 + mc][:],
                                            wt_t[k][:, oi * 128:(oi + 1) * 128],
                                            xt[mh][k][:, mc * 512:(mc + 1) * 512],
                                            start=(k == 0), stop=(k == KT_ - 1))
                        m0 = mh * 1024
                        for oi in range(2):
                            bcol = bias_sb[:, og * 2 + oi:og * 2 + oi + 1]
                            for mc in range(2):
                                ot = opool.tile([128, 512], F32, name="o", tag="o")
                                nc.scalar.activation(
                                    ot[:], ps[oi * 2 + mc][:], copy_f, bias=bcol)
                                if no_out:
                                    nc.sync.dma_start(
                                        outT[og0 + oi * 128:og0 + (oi + 1) * 128,
                                             m0 + mc * 512:m0 + mc * 512 + 4],
                                        ot[:, :4])
                                else:
                                    nc.sync.dma_start(
                                        outT[og0 + oi * 128:og0 + (oi + 1) * 128,
                                             m0 + mc * 512:m0 + (mc + 1) * 512],
                                        ot[:])
    nc.compile()
    return nc


def _build_d(reps=1, w_bufs=24, o_bufs=8, out_bf16=False,
             no_w_dma=False, no_x_dma=False, no_out=False):
    """Variant D: 1 LDWEIGHTS per 4 matmuls and few, fat DMAs.

    Output groups are 128 channels wide (32 groups); each k-step streams one
    stationary [128,128] W block against 4 x-chunks of 512 tokens (full SEQ
    resident in 4 PSUM banks). W arrives host-packed as wP[og*8+kq] =
    [128, 512] tiles holding 4 consecutive k-blocks for one og (so one DMA
    descriptor feeds 4 LDWEIGHTS); x is two [128,1024] half-tiles per k.
    og0 runs token-half-split so the PE only waits for half of x at start.
    Bias is added by the scalar engine during PSUM eviction."""
    nc = bacc.Bacc("TRN2", target_bir_lowering=False, debug=False)
    xT = nc.dram_tensor("xT", [DIN, SEQ], BF16, kind="ExternalInput")
    wP = nc.dram_tensor("wP", [256, 128, 512], BF16, kind="ExternalInput")
    biasR = nc.dram_tensor("biasR", [128, DOUT // 128], F32, kind="ExternalInput")
    out_dt = BF16 if out_bf16 else F32
    outT = nc.dram_tensor("outT", [DOUT, SEQ], out_dt, kind="ExternalOutput")

    KT_ = DIN // 128           # 32 contraction tiles
    OG = DOUT // 128           # 32 output groups of 128 channels
    MC = SEQ // 512            # 4 moving chunks of 512 tokens
    KQ = KT_ // 4              # 8 packed W tiles per og
    copy_f = mybir.ActivationFunctionType.Identity

    with tile.TileContext(nc) as tc:
        with (
            tc.tile_pool(name="xr", bufs=2 * KT_) as xpool,
            tc.tile_pool(name="wt", bufs=w_bufs) as wpool,
            tc.tile_pool(name="bias", bufs=1) as bpool,
            tc.tile_pool(name="outp", bufs=o_bufs) as opool,
            tc.tile_pool(name="psum", bufs=8, space="PSUM") as ppool,
        ):
            bias_sb = bpool.tile([128, DOUT // 128], F32, tag="bias")
            nc.sync.dma_start(bias_sb[:], biasR[:, :])

            def load_wq(og, kq):
                w = wpool.tile([128, 512], BF16, name="w", tag="w")
                if no_w_dma:
                    nc.sync.dma_start(w[:, :4], wP[og * KQ + kq, :, :4])
                else:
                    nc.sync.dma_start(w[:], wP[og * KQ + kq, :, :])
                return w

            def load_x(k, mh):
                # Act-queue (qActDynamicHW): x and out live on the second
                # HWDGE queue so they never sit behind the W stream (SP).
                t = xpool.tile([128, 1024], BF16, name="x", tag="x")
                if no_x_dma:
                    nc.scalar.dma_start(t[:, :4], xT[k * 128:(k + 1) * 128, :4])
                else:
                    nc.scalar.dma_start(
                        t[:], xT[k * 128:(k + 1) * 128,
                                 mh * 1024:(mh + 1) * 1024])
                return t

            def lhsT(wt_t, k):
                return wt_t[k // 4][:, (k % 4) * 128:(k % 4) * 128 + 128]

            def xchunk(xt, k, mc):
                return xt[mc // 2][k][:, (mc % 2) * 512:(mc % 2) * 512 + 512]

            import contextlib
            rep_ctx = tc.For_i(0, reps, 1) if reps > 1 else contextlib.nullcontext()
            with rep_ctx:
                # DMA issue order = consumption order: og0 W + x half 0
                # interleaved (PE chases the stream), then og1 W + x half 1.
                wt_first = [[], []]
                xt = [[None] * KT_, [None] * KT_]
                for kq in range(KQ):
                    wt_first[0].append(load_wq(0, kq))
                    for j in range(4):
                        xt[0][kq * 4 + j] = load_x(kq * 4 + j, 0)
                for kq in range(KQ):
                    wt_first[1].append(load_wq(1, kq))
                    for j in range(4):
                        xt[1][kq * 4 + j] = load_x(kq * 4 + j, 1)
                for og in range(OG):
                    og0 = og * 128
                    wt_t = (wt_first[og] if og < 2 else
                            [load_wq(og, kq) for kq in range(KQ)])
                    bcol = bias_sb[:, og:og + 1]

                    def evict(ps_t, c0):
                        ot = opool.tile([128, 512], out_dt, name="o", tag="o")
                        nc.scalar.activation(ot[:], ps_t[:], copy_f, bias=bcol)
                        if no_out:
                            nc.scalar.dma_start(
                                outT[og0:og0 + 128, c0:c0 + 4], ot[:, :4])
                        else:
                            nc.scalar.dma_start(
                                outT[og0:og0 + 128, c0:c0 + 512], ot[:])

                    if og == 0:
                        # Token-half split: first iteration needs only half
                        # of x, keeping the PE fed during the x stream.
                        for mh in range(2):
                            ps = [ppool.tile([128, 512], F32, name="ps",
                                             tag="ps") for _ in range(2)]
                            for k in range(KT_):
                                for mc in range(2):
                                    nc.tensor.matmul(
                                        ps[mc][:], lhsT(wt_t, k),
                                        xchunk(xt, k, mh * 2 + mc),
                                        start=(k == 0), stop=(k == KT_ - 1))
                            for mc in range(2):
                                evict(ps[mc], mh * 1024 + mc * 512)
                        continue
                    ps = [ppool.tile([128, 512], F32, name="ps", tag="ps")
                          for _ in range(MC)]
                    for k in range(KT_):
                        for mc in range(MC):
                            nc.tensor.matmul(
                                ps[mc][:], lhsT(wt_t, k),
                                xchunk(xt, k, mc),
                                start=(k == 0), stop=(k == KT_ - 1))
                    for mc in range(MC):
                        evict(ps[mc], mc * 512)
    nc.compile()
    return nc


def _pack_w(w_bf16):
    """[DIN, DOUT] bf16 -> wP [256, 128, 512]: wP[og*8+kq, p, j*128+c] =
    W[(kq*4+j)*128 + p, og*128 + c]."""
    arr = np.asarray(w_bf16).reshape(8, 4, 128, 32, 128)  # [kq, j, p, og, c]
    return np.ascontiguousarray(
        arr.transpose(3, 0, 2, 1, 4).reshape(256, 128, 512))


def _f32_to_bf16(a):
    """Round-to-nearest-even fp32 -> bf16 via integer ops (fast, vectorized)."""
    import ml_dtypes
    u = np.ascontiguousarray(a, dtype=np.float32).view(np.uint32)
    r = ((u >> 16) & 1) + np.uint32(0x7FFF)
    return ((u + r) >> 16).astype(np.uint16).view(ml_dtypes.bfloat16)


def _prep_in_maps(x, A, B, weight, bias):
    x = np.asarray(x, dtype=np.float32)
    A = np.asarray(A, dtype=np.float32)
    B = np.asarray(B, dtype=np.float32)
    weight = np.asarray(weight, dtype=np.float32)
    bias = np.asarray(bias, dtype=np.float32)

    wT = _f32_to_bf16(weight.T + A @ B)                      # [DIN, DOUT] bf16
    wP = _pack_w(wT)                                         # [256, 128, 512]
    biasR = np.ascontiguousarray(bias.reshape(DOUT // 128, 128).T)  # [128, 32]

    in_maps = []
    for b in range(N_CORES):
        xTb = _f32_to_bf16(x[b].T)                           # [DIN, SEQ] bf16
        in_maps.append({"xT": xTb, "wT": wT, "wP": wP, "biasR": biasR})
    return in_maps


VARIANT = "D"


def kernel(x, A, B, weight, bias):
    if not _nc_cache:
        _nc_cache.append(
            {"A": _build, "B": _build_b, "C": _build_c,
             "D": _build_d}[VARIANT]())
    nc = _nc_cache[0]

    if VARIANT in ("C", "D"):
        in_maps = _prep_in_maps(x, A, B, weight, bias)
        res = run_bass_kernel_spmd(nc, in_maps, core_ids=list(range(N_CORES)))
        last_result.clear()
        last_result.append(res)
        return np.stack(
            [np.ascontiguousarray(r["outT"].T) for r in res.results], axis=0)

    x = np.asarray(x, dtype=np.float32)
    A = np.asarray(A, dtype=np.float32)
    B = np.asarray(B, dtype=np.float32)
    weight = np.asarray(weight, dtype=np.float32)
    bias = np.asarray(bias, dtype=np.float32)

    wT = np.ascontiguousarray(weight.T)                      # [DIN, DOUT]
    bB = np.concatenate([B, bias[None, :]], axis=0)          # [RANK+1, DOUT]
    bB = np.ascontiguousarray(bB, dtype=np.float32)

    in_maps = []
    for b in range(N_CORES):
        xb = x[b]                                            # [SEQ, DIN]
        xTb = np.ascontiguousarray(xb.T)                     # [DIN, SEQ]
        xa = xb @ A                                          # [SEQ, RANK]
        xaT = np.concatenate(
            [np.ascontiguousarray(xa.T),
             np.ones((1, SEQ), dtype=np.float32)], axis=0)   # [RANK+1, SEQ]
        in_maps.append({"xT": xTb, "wT": wT, "xaT": xaT, "bB": bB})

    res = run_bass_kernel_spmd(nc, in_maps, core_ids=list(range(N_CORES)))
    last_result.clear()
    last_result.append(res)
    if VARIANT == "B":
        return np.stack(
            [np.ascontiguousarray(r["outT"].T) for r in res.results], axis=0)
    return np.stack([r["out"] for r in res.results], axis=0)


last_result = []

